# revision 35
# baseline (speedup 1.0000x reference)
"""Trainium2 Bass kernel for nn_BackwardTransformLayer (inverse DWT synthesis step).

Math: out[r, 2j+s] = sum_{p=0..3} g[2p+s]*d[r,(j+p+s')%M] + h[2p+s]*a[r,...]
  (g = flip(scaling) with odd idx negated; h = scaling; even outputs use
   shifts 0..3 of taps g[0,2,4,6], odd outputs shifts 1..4 of g[1,3,5,7])

Strategy (8 cores data-parallel over rows, 512 rows/core):
  - fp16 end-to-end on the wire: the host casts inputs f32->f16 and the
    kernel writes f16 output (cast back to f32 on host). This halves HBM
    traffic vs f32 (64 MiB -> 32 MiB per core), moving the bandwidth floor
    from ~183 us to ~93 us. fp16 quantization error (~3e-4 RMS) is far
    below the 2e-2 correctness gate.
  - The polyphase stencil along columns is a banded linear operator: for each
    128-column input block k, out[:, 256k:256k+256] = d_blk @ W_d + a_blk @ W_a
    plus a tiny "halo" contribution from the first 4 columns of block k+1
    (circularly wrapped) hitting output columns 249..255 of the chunk.
  - TensorE computes the banded products with stationary = PE-transposed input
    tile dT[incol, row] and moving = W[incol, outcol]; results land naturally
    oriented [row, outcol] in PSUM (fp32 accum). fp16 matmul/transpose run at
    1 cyc/row, so PE (~83 us/core) stays under the DMA floor.
  - Halo contributions are NOT matmuls (partial-PSUM-write matmuls measured
    catastrophically slow): they are strided scalar_tensor_tensor MACs on the
    SBUF output strip, reading input strips that carry 4 extra wrapped
    columns so the stride is uniform across all chunks of a half.
  - DVE copies transposed tiles PSUM->SBUF and does halo MACs; ACT copies
    finished output chunk-pairs PSUM->SBUF (casting f32->f16); all DMA is
    contiguous and cast-free.

  Schedule (cost-model-sim tuned; per-core DMA queue runs gap-free):
  - Engines are rate-matched per 128-row group (~20 us each on PE/DVE/ACT,
    ~23 us DMA) - classic ridge. All ordering serves the DMA queue.
  - Inputs load as whole 2.1 MB strips, one DMA per (stream, group); group
    0 splits into ascending pieces (d/a interleaved) so PE starts ~2.5 us
    in. Loads for group g+2 are emitted right after group g's compute, ahead
    of group g's stores in SP FIFO order.
  - A group's halo MACs + stores are DEFERRED and interleaved through the
    NEXT group's quad loop: DVE never runs a solid MAC block that would
    stall PE's quad-copy feed, and each store issues as soon as its half is
    patched. The last group drains at shrinking granularity (16/16 and
    16/8/8 chunk sub-stores) to keep the final copy->MAC->store chain short.
  - Wrapped halo columns come from an SBUF->SBUF copy (cols 0:4 of the same
    strip), not a tiny DMA; all consts ship in one DMA.

Measured (8 cores, min-slope of on-device repeat loop): ~102.4 us vs
183.4 us f32 baseline (1.79x); cost-model sim predicts 103.3 us with the
DMA device busy 0->101.6 us with zero gaps.

Env:
  BASS_IO16=1 (default) fp16 wire dtype; =0 f32 wire dtype (debug only; use
    BASS_INBUFS=2 BASS_OUTBUFS=2 to fit SBUF).
  BASS_MM_F32R=1 (default) f32-path matmuls in float32r; =0 exact fp32.
  BASS_DMA_SPLIT=1 issue output stores on the scalar-engine HWDGE ring
    (measured neutral-to-worse; default off).
  BASS_ABLATE=dma|nohalo timing ablations (wrong results by design).
"""

import os
import sys
from contextlib import ExitStack

import numpy as np

sys.path.insert(0, "/opt/trn_rl_repo")

import concourse.bass as bass  # noqa: E402
import concourse.mybir as mybir  # noqa: E402
import concourse.tile as tile  # noqa: E402
from concourse import bacc  # noqa: E402
from concourse.bass_utils import run_bass_kernel_spmd  # noqa: E402

N_CORES = 8
N_ROWS = 4096
M = 8192  # input columns per row
PG = 128  # rows per group (partition dim)
BLK = 128  # input columns per block
OUTW = 2 * BLK  # output columns per chunk
HALF = M // 2  # input columns per half-strip
NBLK_HALF = HALF // BLK  # 32 blocks per half-strip
NBLK = M // BLK  # 64 blocks
EXTW = HALF + BLK  # extended strip width (one extra block; 4 cols used)
F32 = mybir.dt.float32
F32R = mybir.dt.float32r
F16 = mybir.dt.float16

IO16 = os.environ.get("BASS_IO16", "1") == "1"
IO_DT = F16 if IO16 else F32
NP_IO = np.float16 if IO16 else np.float32
MM_F32R = os.environ.get("BASS_MM_F32R", "1") == "1"
DMA_SPLIT = os.environ.get("BASS_DMA_SPLIT", "0") == "1"
BATCHED = os.environ.get("BASS_BATCH", "0") == "1"

_BUILD_CACHE = {}


def _halo_positions():
    """Static (stream, kp, n, tap) positions of halo coefficients.

    Chunk outcol n (0..255) gets a contribution coeff[tap] * x[:, 128*(k+1)+kp]
    from the next block's first 4 columns.
    """
    pos = []
    for sti in range(2):  # 0 = details (g), 1 = approximation (h)
        for v in range(128):
            for s in range(4):
                kp = v + s - 128
                if 0 <= kp <= 3:
                    pos.append((sti, kp, 2 * v, 2 * s))
                kp2 = v + 1 + s - 128
                if 0 <= kp2 <= 3:
                    pos.append((sti, kp2, 2 * v + 1, 2 * s + 1))
    return pos


HALO_POS = _halo_positions()  # 32 entries


def _build_weights(scaling: np.ndarray):
    h = np.asarray(scaling, dtype=np.float32)
    g = h[::-1].copy()
    g[1::2] *= -1.0

    def build_main(f):
        W = np.zeros((BLK, OUTW), np.float32)
        for k in range(BLK):
            for v in range(BLK):
                s = k - v
                if 0 <= s <= 3:
                    W[k, 2 * v] = f[2 * s]
                s = k - v - 1
                if 0 <= s <= 3:
                    W[k, 2 * v + 1] = f[2 * s + 1]
        return W

    hvec = np.zeros((128, len(HALO_POS)), np.float32)
    for i, (sti, kp, n, tap) in enumerate(HALO_POS):
        hvec[:, i] = (g if sti == 0 else h)[tap]

    wd, wa = build_main(g), build_main(h)
    if IO16:
        return wd.astype(np.float16), wa.astype(np.float16), hvec.astype(np.float16)
    return wd, wa, hvec


def _build(rows_per_core: int, mm_f32r: bool, repeat: int = 1, ablate: str = None):
    if ablate is None:
        ablate = os.environ.get("BASS_ABLATE", "")
    key = (rows_per_core, mm_f32r, repeat, ablate, IO16)
    if key in _BUILD_CACHE:
        return _BUILD_CACHE[key]

    ngroups = rows_per_core // PG
    mm_dt = F16 if IO16 else (F32R if mm_f32r else F32)
    # PSUM tile dtype for PE transposes: match input dtype in fp16 mode
    pt_dt = F16 if IO16 else F32

    nc = bacc.Bacc("TRN2", target_bir_lowering=False, debug=False)
    d_dram = nc.dram_tensor("details", [rows_per_core, M], IO_DT, kind="ExternalInput").ap()
    a_dram = nc.dram_tensor("approximation", [rows_per_core, M], IO_DT, kind="ExternalInput").ap()
    if IO16:
        # single const tensor: [ident | w_d | w_a | hvec] along free dim
        cw = 128 + OUTW + OUTW + len(HALO_POS)
        c_dram = nc.dram_tensor("consts", [128, cw], IO_DT, kind="ExternalInput").ap()
    else:
        wd_dram = nc.dram_tensor("w_d", [BLK, OUTW], mm_dt, kind="ExternalInput").ap()
        wa_dram = nc.dram_tensor("w_a", [BLK, OUTW], mm_dt, kind="ExternalInput").ap()
        hv_dram = nc.dram_tensor("w_hvec", [128, len(HALO_POS)], IO_DT, kind="ExternalInput").ap()
        id_dram = nc.dram_tensor("ident", [128, 128], IO_DT, kind="ExternalInput").ap()
    out_dram = nc.dram_tensor("out", [rows_per_core, 2 * M], IO_DT, kind="ExternalOutput").ap()

    store_eng = nc.scalar if DMA_SPLIT else nc.sync

    # input strip generations live simultaneously: g-1 (deferred MACs),
    # g (computing), g+1 (loaded), g+2 (loading)
    inbufs = int(os.environ.get("BASS_INBUFS", "0")) or min(ngroups, 4)

    with tile.TileContext(nc) as tc, ExitStack() as ctx:
        const = ctx.enter_context(tc.tile_pool(name="const", bufs=1))
        inp = ctx.enter_context(tc.tile_pool(name="inp", bufs=inbufs))
        tq = ctx.enter_context(tc.tile_pool(name="tq", bufs=16 if BATCHED else 3))
        outp = ctx.enter_context(
            tc.tile_pool(name="outp", bufs=int(os.environ.get("BASS_OUTBUFS", "4")))
        )
        halo = ctx.enter_context(tc.tile_pool(name="halo", bufs=2))
        ps_t = ctx.enter_context(tc.tile_pool(name="ps_t", bufs=3, space="PSUM"))
        ps_o = ctx.enter_context(
            tc.tile_pool(name="ps_o", bufs=int(os.environ.get("BASS_PSOBUFS", "5")), space="PSUM")
        )

        if IO16:
            const_s = const.tile([128, cw], IO_DT)
            nc.sync.dma_start(const_s[:], c_dram)
            ident_s = const_s[:, 0:128]
            wd_s = const_s[:, 128 : 128 + OUTW]
            wa_s = const_s[:, 128 + OUTW : 128 + 2 * OUTW]
            hv_s = const_s[:, 128 + 2 * OUTW : cw]
        else:
            ident_t = const.tile([128, 128], IO_DT)
            nc.sync.dma_start(ident_t[:], id_dram)
            wd_t = const.tile([BLK, OUTW], mm_dt)
            nc.sync.dma_start(wd_t[:], wd_dram)
            wa_t = const.tile([BLK, OUTW], mm_dt)
            nc.sync.dma_start(wa_t[:], wa_dram)
            hv_t = const.tile([128, len(HALO_POS)], IO_DT)
            nc.sync.dma_start(hv_t[:], hv_dram)
            ident_s, wd_s, wa_s, hv_s = ident_t[:], wd_t[:], wa_t[:], hv_t[:]

        if os.environ.get("BASS_WARMUP", "1") == "1":
            # ~4.3us of dummy PE work at kernel start, hidden under the first
            # input DMA: trips the HAM activity window so the first real
            # transposes/matmuls run at 2.4 GHz instead of the cold 1.2 GHz.
            warm = ps_t.tile([128, 128], F32, tag="ps_t", name="warm")
            for _ in range(10):
                nc.tensor.matmul(warm[:], ident_s, ident_s, start=True,
                                 stop=True, skip_group_check=True)

        # input strip tile: full row (M cols) + 4 wrap cols + pad so each
        # half-view [hh*HALF : hh*HALF + HALF+BLK] is rearrangeable
        STRIPW = HALF + EXTW  # 8320

        def emit_loads(grp):
            """Issue the input DMAs for one group; return {stream: strip}.

            Groups load the whole 2.1MB strip in one DMA (best transfer
            efficiency); group 0 splits h0/h1 with h0 for BOTH streams first
            so PE starts ~5us earlier.
            """
            r0 = grp * PG
            strips = {}
            if grp == 0:
                for st, dram in (("d", d_dram), ("a", a_dram)):
                    strips[st] = inp.tile([PG, STRIPW], IO_DT, tag=f"in_{st}",
                                          name=f"in_{st}_g{grp}")
                # ascending-size pieces, d/a interleaved: PE's first quads
                # can start ~2.5us in instead of waiting for a full strip
                for lo, hi in ((0, 1024), (1024, HALF + 4), (HALF + 4, M)):
                    for st, dram in (("d", d_dram), ("a", a_dram)):
                        nc.sync.dma_start(
                            strips[st][:, lo:hi], dram[r0 : r0 + PG, lo:hi]
                        )
            else:
                for st, dram in (("d", d_dram), ("a", a_dram)):
                    t = inp.tile([PG, STRIPW], IO_DT, tag=f"in_{st}",
                                 name=f"in_{st}_g{grp}")
                    nc.sync.dma_start(t[:, 0:M], dram[r0 : r0 + PG, 0:M])
                    strips[st] = t
            return strips

        def emit_group(grp, strips, deferred_prev, fine=False):
            """Emit transposes/matmuls/PSUM-drains for grp, interleaving the
            PREVIOUS group's deferred halo MACs + stores through the quad
            loop (so DVE never runs a solid MAC block that stalls PE, and
            stores issue as soon as their half is patched). Returns this
            group's deferred op list."""
            r0 = grp * PG
            out_halves = [
                outp.tile([PG, 2 * HALF], IO_DT, tag="out", name=f"out_g{grp}h{i}")
                for i in range(2)
            ]

            if ablate == "dma":
                for op in deferred_prev:
                    op()
                for hh in range(2):
                    nc.vector.tensor_copy(
                        out=out_halves[hh][:, 0:1],
                        in_=strips["d"][:, hh * HALF : hh * HALF + 1],
                    )
                    store_eng.dma_start(
                        out_dram[r0 : r0 + PG, hh * 2 * HALF : (hh + 1) * 2 * HALF],
                        out_halves[hh][:],
                    )
                return []

            quads = {"d": [], "a": []}

            def make_quad(st, q):
                blocks = [4 * q + i for i in range(4)]
                pt = ps_t.tile([128, 512], pt_dt, tag="ps_t", name=f"pt_{st}{q}")
                for i, b in enumerate(blocks):
                    nc.tensor.transpose(
                        pt[:, 128 * i : 128 * (i + 1)],
                        strips[st][:, b * BLK : (b + 1) * BLK],
                        ident_s,
                    )
                qt = tq.tile([128, 512], mm_dt, tag=f"tq_{st}", name=f"qt_{st}{q}")
                nc.vector.tensor_copy(out=qt[:], in_=pt[:])
                quads[st].append(qt)

            def make_chunk_pair(t):
                # chunks k=2t, 2t+1 share one PSUM bank and one ACT copy
                po = ps_o.tile([128, 2 * OUTW], F32, tag="ps_o", name=f"po_{t}")
                for half_idx in range(2):
                    k = 2 * t + half_idx
                    q, off = divmod(k, 4)
                    lhs_d = quads["d"][q][:, off * 128 : off * 128 + 128]
                    lhs_a = quads["a"][q][:, off * 128 : off * 128 + 128]
                    sl = po[:, half_idx * OUTW : (half_idx + 1) * OUTW]
                    nc.tensor.matmul(sl, lhs_d, wd_s, start=True, stop=False,
                                     skip_group_check=True)
                    nc.tensor.matmul(sl, lhs_a, wa_s, start=False, stop=True,
                                     skip_group_check=True)
                hh, tt = divmod(t, NBLK_HALF // 2)
                nc.scalar.copy(
                    out=out_halves[hh][:, tt * 2 * OUTW : (tt + 1) * 2 * OUTW],
                    in_=po[:],
                )

            # --- halo via compact PATCH tiles --------------------------------
            # patch[p, j, c] = sum over (stream, kp) of x[p, 128(c+1)+kp]*coef
            # for output column n = 249+j of chunk c. Building it reads the
            # gathered xh tiles (8B strides) and writes contiguous 32-elem
            # runs. The old direct MACs into the output strip (512B-stride
            # single columns, 96 ops/group) measured ~26us of exposed DVE
            # time on HW; here only the two final adds touch the strided
            # output view, and patch-building depends ONLY on input strips so
            # it hides anywhere in the group.
            NJ = 7  # patched output columns per chunk: n in [249, 255]

            patches = {}
            xhs = {}

            def halo_build_ops():
                """Ops needing only the input strips; spread through this
                group's own quad loop."""
                if ablate == "nohalo":
                    return []
                ops = []
                for st in ("d", "a"):
                    def wrap(st=st):
                        nc.vector.tensor_copy(
                            out=strips[st][:, M : M + 4], in_=strips[st][:, 0:4]
                        )
                    ops.append(wrap)
                for hh in range(2):
                    for st in ("d", "a"):
                        xh = halo.tile(
                            [128, NBLK_HALF * 4], IO_DT, tag=f"xh_{st}{hh}",
                            name=f"xh_{st}{hh}_g{grp}",
                        )
                        xhs[(st, hh)] = xh

                        def gather(st=st, hh=hh, xh=xh):
                            nc.vector.tensor_copy(
                                out=xh[:].rearrange("p (c k) -> p c k", k=4),
                                in_=strips[st][:, hh * HALF : hh * HALF + EXTW]
                                .rearrange("p (c w) -> p c w", w=BLK)
                                [:, 1 : NBLK_HALF + 1, 0:4],
                            )
                        ops.append(gather)
                    p = halo.tile(
                        [128, NJ * NBLK_HALF], IO_DT, tag=f"patch{hh}",
                        name=f"patch{hh}_g{grp}",
                    )
                    patches[hh] = p

                    def ms(p=p):
                        nc.vector.memset(p[:], 0.0)
                    ops.append(ms)
                    for i, (sti, kp, n, tap) in enumerate(HALO_POS):
                        st = "d" if sti == 0 else "a"
                        j = n - 249

                        def mac(hh=hh, i=i, kp=kp, j=j, st=st, p=p):
                            pr = p[:].rearrange("p (j c) -> p j c", j=NJ)
                            xv = xhs[(st, hh)][:].rearrange(
                                "p (c k) -> p k c", k=4
                            )
                            o = pr[:, j : j + 1, :]
                            nc.vector.scalar_tensor_tensor(
                                out=o,
                                in0=xv[:, kp : kp + 1, :],
                                scalar=hv_s[:, i : i + 1],
                                in1=o,
                                op0=mybir.AluOpType.mult,
                                op1=mybir.AluOpType.add,
                            )
                        ops.append(mac)
                return ops

            def make_deferred():
                """Tail ops: final patch adds (after the half's ACT copies)
                and stores. Fine mode (last group) shrinks the chunks so the
                final copy->add->store drain chain is short."""
                ops = []

                def add_op(hh, c0, c1):
                    p = patches[hh]

                    def op():
                        oh3 = out_halves[hh][:].rearrange(
                            "p (c w) -> p c w", w=OUTW
                        )
                        o = oh3[:, c0:c1, 249 : 249 + NJ]
                        pt = p[:].rearrange("p (j c) -> p c j", j=NJ)
                        nc.vector.scalar_tensor_tensor(
                            out=o,
                            in0=pt[:, c0:c1, :],
                            scalar=1.0,
                            in1=o,
                            op0=mybir.AluOpType.mult,
                            op1=mybir.AluOpType.add,
                        )
                    return op

                def store_op(hh, c0, c1):
                    def op():
                        store_eng.dma_start(
                            out_dram[
                                r0 : r0 + PG,
                                hh * 2 * HALF + c0 * OUTW : hh * 2 * HALF
                                + c1 * OUTW,
                            ],
                            out_halves[hh][:, c0 * OUTW : c1 * OUTW],
                        )
                    return op

                if fine:
                    subs = {0: [(0, 16), (16, 32)], 1: [(0, 16), (16, 24), (24, 32)]}
                else:
                    subs = {0: [(0, 32)], 1: [(0, 32)]}
                for hh in range(2):
                    for c0, c1 in subs[hh]:
                        if ablate != "nohalo":
                            ops.append(add_op(hh, c0, c1))
                        ops.append(store_op(hh, c0, c1))
                return ops

            own_ops = halo_build_ops()
            nsteps = NBLK // 4  # 16 quad steps
            np_prev = len(deferred_prev)
            n_own = len(own_ops)
            for q in range(nsteps):
                make_quad("d", q)
                make_quad("a", q)
                for t in range(2 * q, 2 * q + 2):
                    make_chunk_pair(t)
                # previous group's tail ops first (stores issue early), then
                # this group's halo-build ops
                for op in deferred_prev[
                    np_prev * q // nsteps : np_prev * (q + 1) // nsteps
                ]:
                    op()
                for op in own_ops[n_own * q // nsteps : n_own * (q + 1) // nsteps]:
                    op()

            return make_deferred()

        def emit_all():
            # primed interleave: 2 groups of loads run ahead of compute, and
            # each later group's loads are emitted BEFORE the previous group's
            # stores so a store's sem-wait never head-of-line-blocks a load on
            # the SP queue.
            prime = min(2, ngroups)
            pending = {g: emit_loads(g) for g in range(prime)}
            deferred = []
            for g in range(ngroups):
                # NOTE: loads for g+prime are emitted AFTER emit_group(g) so
                # that group g-1's deferred MACs (emitted inside emit_group(g))
                # are already recorded as consumers of the input tiles that
                # these loads recycle — otherwise the load would skip that WAR
                # dependency and clobber a strip the MACs still read.
                deferred = emit_group(
                    g, pending.pop(g), deferred, fine=(g == ngroups - 1)
                )
                nxt = g + prime
                if nxt < ngroups:
                    pending[nxt] = emit_loads(nxt)
            # drain the last group's halo MACs + stores
            for op in deferred:
                op()

        if repeat > 1:
            with tc.For_i(0, repeat, 1):
                emit_all()
        else:
            emit_all()

    nc.compile()
    _BUILD_CACHE[key] = nc
    return nc


def _make_consts(scaling):
    """Host-side constants keyed by dram tensor name."""
    wd, wa, hvec = _build_weights(scaling)
    ident = np.eye(128, dtype=NP_IO)
    if IO16:
        return {"consts": np.concatenate([ident, wd, wa, hvec], axis=1)}
    return {"w_d": wd, "w_a": wa, "w_hvec": hvec, "ident": ident}


def _run(details, approximation, scaling, rows_per_core, core_ids, mm_f32r, **kw):
    consts = _make_consts(scaling)
    nc = _build(rows_per_core, mm_f32r)
    details = np.asarray(details, dtype=NP_IO)
    approximation = np.asarray(approximation, dtype=NP_IO)
    in_maps = []
    for c in core_ids:
        r0 = c * rows_per_core
        m = {
            "details": np.ascontiguousarray(details[r0 : r0 + rows_per_core]),
            "approximation": np.ascontiguousarray(
                approximation[r0 : r0 + rows_per_core]
            ),
        }
        m.update(consts)
        in_maps.append(m)
    res = run_bass_kernel_spmd(nc, in_maps, core_ids=list(range(len(core_ids))), **kw)
    out = np.concatenate([res.results[i]["out"] for i in range(len(core_ids))], axis=0)
    return out, res


def kernel(details, approximation, scaling):
    details = np.asarray(details, dtype=np.float32)
    approximation = np.asarray(approximation, dtype=np.float32)
    scaling = np.asarray(scaling, dtype=np.float32)
    rows_per_core = details.shape[0] // N_CORES
    out, _ = _run(
        details, approximation, scaling, rows_per_core, list(range(N_CORES)),
        MM_F32R,
    )
    return np.asarray(out, dtype=np.float32)


# revision 43
# speedup vs baseline: 1.2526x; 1.2526x over previous
"""Trainium2 Bass kernel for nn_BackwardTransformLayer (inverse DWT synthesis step).

Math: out[r, 2j+s] = sum_{p=0..3} g[2p+s]*d[r,(j+p+s')%M] + h[2p+s]*a[r,...]
  (g = flip(scaling) with odd idx negated; h = scaling; even outputs use
   shifts 0..3 of taps g[0,2,4,6], odd outputs shifts 1..4 of g[1,3,5,7])

Strategy (8 cores data-parallel over rows, 512 rows/core):
  - fp16 end-to-end on the wire: the host casts inputs f32->f16 and the
    kernel writes f16 output (cast back to f32 on host). This halves HBM
    traffic vs f32 (64 MiB -> 32 MiB per core), moving the bandwidth floor
    from ~183 us to ~93 us. fp16 quantization error (~3e-4 RMS) is far
    below the 2e-2 correctness gate.
  - The polyphase stencil along columns is a banded linear operator: for each
    128-column input block k, out[:, 256k:256k+256] = d_blk @ W_d + a_blk @ W_a
    plus a tiny "halo" contribution from the first 4 columns of block k+1
    (circularly wrapped) hitting output columns 249..255 of the chunk.
  - TensorE computes the banded products with stationary = PE-transposed input
    tile dT[incol, row] and moving = W[incol, outcol]; results land naturally
    oriented [row, outcol] in PSUM (fp32 accum). fp16 matmul/transpose run at
    1 cyc/row, so PE (~83 us/core) stays under the DMA floor.
  - Halo contributions are NOT matmuls (partial-PSUM-write matmuls measured
    catastrophically slow): they are strided scalar_tensor_tensor MACs on the
    SBUF output strip, reading input strips that carry 4 extra wrapped
    columns so the stride is uniform across all chunks of a half.
  - DVE copies transposed tiles PSUM->SBUF and does halo MACs; ACT copies
    finished output chunk-pairs PSUM->SBUF (casting f32->f16); all DMA is
    contiguous and cast-free.

  Schedule (cost-model-sim tuned; per-core DMA queue runs gap-free):
  - Engines are rate-matched per 128-row group (~20 us each on PE/DVE/ACT,
    ~23 us DMA) - classic ridge. All ordering serves the DMA queue.
  - Inputs load as whole 2.1 MB strips, one DMA per (stream, group); group
    0 splits into ascending pieces (d/a interleaved) so PE starts ~2.5 us
    in. Loads for group g+2 are emitted right after group g's compute, ahead
    of group g's stores in SP FIFO order.
  - A group's halo MACs + stores are DEFERRED and interleaved through the
    NEXT group's quad loop: DVE never runs a solid MAC block that would
    stall PE's quad-copy feed, and each store issues as soon as its half is
    patched. The last group drains at shrinking granularity (16/16 and
    16/8/8 chunk sub-stores) to keep the final copy->MAC->store chain short.
  - Wrapped halo columns come from an SBUF->SBUF copy (cols 0:4 of the same
    strip), not a tiny DMA; all consts ship in one DMA.

Measured (8 cores, min-slope of on-device repeat loop): ~102.4 us vs
183.4 us f32 baseline (1.79x); cost-model sim predicts 103.3 us with the
DMA device busy 0->101.6 us with zero gaps.

Env:
  BASS_IO16=1 (default) fp16 wire dtype; =0 f32 wire dtype (debug only; use
    BASS_INBUFS=2 BASS_OUTBUFS=2 to fit SBUF).
  BASS_MM_F32R=1 (default) f32-path matmuls in float32r; =0 exact fp32.
  BASS_DMA_SPLIT=1 issue output stores on the scalar-engine HWDGE ring
    (measured neutral-to-worse; default off).
  BASS_ABLATE=dma|nohalo timing ablations (wrong results by design).
"""

import os
import sys
from contextlib import ExitStack

import numpy as np

sys.path.insert(0, "/opt/trn_rl_repo")

import concourse.bass as bass  # noqa: E402
import concourse.mybir as mybir  # noqa: E402
import concourse.tile as tile  # noqa: E402
from concourse import bacc  # noqa: E402
from concourse.bass_utils import run_bass_kernel_spmd  # noqa: E402

N_CORES = 8
N_ROWS = 4096
M = 8192  # input columns per row
PG = 128  # rows per group (partition dim)
BLK = 128  # input columns per block
OUTW = 2 * BLK  # output columns per chunk
HALF = M // 2  # input columns per half-strip
NBLK_HALF = HALF // BLK  # 32 blocks per half-strip
NBLK = M // BLK  # 64 blocks
EXTW = HALF + BLK  # extended strip width (one extra block; 4 cols used)
F32 = mybir.dt.float32
F32R = mybir.dt.float32r
F16 = mybir.dt.float16

IO16 = os.environ.get("BASS_IO16", "1") == "1"
IO_DT = F16 if IO16 else F32
NP_IO = np.float16 if IO16 else np.float32
MM_F32R = os.environ.get("BASS_MM_F32R", "1") == "1"
DMA_SPLIT = os.environ.get("BASS_DMA_SPLIT", "0") == "1"
BATCHED = os.environ.get("BASS_BATCH", "0") == "1"

_BUILD_CACHE = {}


def _halo_positions():
    """Static (stream, kp, n, tap) positions of halo coefficients.

    Chunk outcol n (0..255) gets a contribution coeff[tap] * x[:, 128*(k+1)+kp]
    from the next block's first 4 columns.
    """
    pos = []
    for sti in range(2):  # 0 = details (g), 1 = approximation (h)
        for v in range(128):
            for s in range(4):
                kp = v + s - 128
                if 0 <= kp <= 3:
                    pos.append((sti, kp, 2 * v, 2 * s))
                kp2 = v + 1 + s - 128
                if 0 <= kp2 <= 3:
                    pos.append((sti, kp2, 2 * v + 1, 2 * s + 1))
    return pos


HALO_POS = _halo_positions()  # 32 entries


def _build_weights(scaling: np.ndarray):
    h = np.asarray(scaling, dtype=np.float32)
    g = h[::-1].copy()
    g[1::2] *= -1.0

    def build_main(f):
        W = np.zeros((BLK, OUTW), np.float32)
        for k in range(BLK):
            for v in range(BLK):
                s = k - v
                if 0 <= s <= 3:
                    W[k, 2 * v] = f[2 * s]
                s = k - v - 1
                if 0 <= s <= 3:
                    W[k, 2 * v + 1] = f[2 * s + 1]
        return W

    hvec = np.zeros((128, len(HALO_POS)), np.float32)
    for i, (sti, kp, n, tap) in enumerate(HALO_POS):
        hvec[:, i] = (g if sti == 0 else h)[tap]

    # hv2[st, kp, j]: coefficient multiplying x[:, 128(c+1)+kp] into output
    # column n = 249+j of chunk c (0 where no such halo entry exists)
    hv2 = np.zeros((128, 2 * 4 * 7), np.float32)
    for sti, kp, n, tap in HALO_POS:
        hv2[:, (sti * 4 + kp) * 7 + (n - 249)] = (g if sti == 0 else h)[tap]

    wd, wa = build_main(g), build_main(h)
    if IO16:
        return (wd.astype(np.float16), wa.astype(np.float16),
                hvec.astype(np.float16), hv2.astype(np.float16))
    return wd, wa, hvec, hv2


def _build(rows_per_core: int, mm_f32r: bool, repeat: int = 1, ablate: str = None):
    if ablate is None:
        ablate = os.environ.get("BASS_ABLATE", "")
    key = (rows_per_core, mm_f32r, repeat, ablate, IO16)
    if key in _BUILD_CACHE:
        return _BUILD_CACHE[key]

    ngroups = rows_per_core // PG
    mm_dt = F16 if IO16 else (F32R if mm_f32r else F32)
    # PSUM tile dtype for PE transposes: match input dtype in fp16 mode
    pt_dt = F16 if IO16 else F32

    nc = bacc.Bacc("TRN2", target_bir_lowering=False, debug=False)
    d_dram = nc.dram_tensor("details", [rows_per_core, M], IO_DT, kind="ExternalInput").ap()
    a_dram = nc.dram_tensor("approximation", [rows_per_core, M], IO_DT, kind="ExternalInput").ap()
    if IO16:
        # single const tensor: [ident | w_d | w_a | hvec | hv2] along free dim
        cw = 128 + OUTW + OUTW + len(HALO_POS) + 56
        c_dram = nc.dram_tensor("consts", [128, cw], IO_DT, kind="ExternalInput").ap()
    else:
        wd_dram = nc.dram_tensor("w_d", [BLK, OUTW], mm_dt, kind="ExternalInput").ap()
        wa_dram = nc.dram_tensor("w_a", [BLK, OUTW], mm_dt, kind="ExternalInput").ap()
        hv_dram = nc.dram_tensor("w_hvec", [128, len(HALO_POS)], IO_DT, kind="ExternalInput").ap()
        hv2_dram = nc.dram_tensor("w_hv2", [128, 56], IO_DT, kind="ExternalInput").ap()
        id_dram = nc.dram_tensor("ident", [128, 128], IO_DT, kind="ExternalInput").ap()
    out_dram = nc.dram_tensor("out", [rows_per_core, 2 * M], IO_DT, kind="ExternalOutput").ap()

    store_eng = nc.scalar if DMA_SPLIT else nc.sync

    # input strip generations live simultaneously: g-1 (deferred MACs),
    # g (computing), g+1 (loaded), g+2 (loading)
    inbufs = int(os.environ.get("BASS_INBUFS", "0")) or min(ngroups, 4)

    with tile.TileContext(nc) as tc, ExitStack() as ctx:
        const = ctx.enter_context(tc.tile_pool(name="const", bufs=1))
        inp = ctx.enter_context(tc.tile_pool(name="inp", bufs=inbufs))
        tq = ctx.enter_context(tc.tile_pool(name="tq", bufs=16 if BATCHED else 3))
        outp = ctx.enter_context(
            tc.tile_pool(name="outp", bufs=int(os.environ.get("BASS_OUTBUFS", "4")))
        )
        halo = ctx.enter_context(tc.tile_pool(name="halo", bufs=2))
        ps_t = ctx.enter_context(tc.tile_pool(name="ps_t", bufs=3, space="PSUM"))
        ps_o = ctx.enter_context(
            tc.tile_pool(name="ps_o", bufs=int(os.environ.get("BASS_PSOBUFS", "5")), space="PSUM")
        )

        if IO16:
            const_s = const.tile([128, cw], IO_DT)
            nc.sync.dma_start(const_s[:], c_dram)
            ident_s = const_s[:, 0:128]
            wd_s = const_s[:, 128 : 128 + OUTW]
            wa_s = const_s[:, 128 + OUTW : 128 + 2 * OUTW]
            hv_s = const_s[:, 128 + 2 * OUTW : cw - 56]
            hv2_s = const_s[:, cw - 56 : cw]
        else:
            ident_t = const.tile([128, 128], IO_DT)
            nc.sync.dma_start(ident_t[:], id_dram)
            wd_t = const.tile([BLK, OUTW], mm_dt)
            nc.sync.dma_start(wd_t[:], wd_dram)
            wa_t = const.tile([BLK, OUTW], mm_dt)
            nc.sync.dma_start(wa_t[:], wa_dram)
            hv_t = const.tile([128, len(HALO_POS)], IO_DT)
            nc.sync.dma_start(hv_t[:], hv_dram)
            hv2_t = const.tile([128, 56], IO_DT)
            nc.sync.dma_start(hv2_t[:], hv2_dram)
            ident_s, wd_s, wa_s, hv_s = ident_t[:], wd_t[:], wa_t[:], hv_t[:]
            hv2_s = hv2_t[:]

        if os.environ.get("BASS_WARMUP", "1") == "1":
            # ~4.3us of dummy PE work at kernel start, hidden under the first
            # input DMA: trips the HAM activity window so the first real
            # transposes/matmuls run at 2.4 GHz instead of the cold 1.2 GHz.
            warm = ps_t.tile([128, 128], F32, tag="ps_t", name="warm")
            for _ in range(10):
                nc.tensor.matmul(warm[:], ident_s, ident_s, start=True,
                                 stop=True, skip_group_check=True)

        # input strip tile: full row (M cols) + 4 wrap cols + pad so each
        # half-view [hh*HALF : hh*HALF + HALF+BLK] is rearrangeable
        STRIPW = HALF + EXTW  # 8320

        def emit_loads(grp):
            """Issue the input DMAs for one group; return {stream: strip}.

            Groups load the whole 2.1MB strip in one DMA (best transfer
            efficiency); group 0 splits h0/h1 with h0 for BOTH streams first
            so PE starts ~5us earlier.
            """
            r0 = grp * PG
            strips = {}
            if grp == 0:
                for st, dram in (("d", d_dram), ("a", a_dram)):
                    strips[st] = inp.tile([PG, STRIPW], IO_DT, tag=f"in_{st}",
                                          name=f"in_{st}_g{grp}")
                # ascending-size pieces, d/a interleaved: PE's first quads
                # can start ~2.5us in instead of waiting for a full strip
                for lo, hi in ((0, 1024), (1024, HALF + 4), (HALF + 4, M)):
                    for st, dram in (("d", d_dram), ("a", a_dram)):
                        nc.sync.dma_start(
                            strips[st][:, lo:hi], dram[r0 : r0 + PG, lo:hi]
                        )
            else:
                for st, dram in (("d", d_dram), ("a", a_dram)):
                    t = inp.tile([PG, STRIPW], IO_DT, tag=f"in_{st}",
                                 name=f"in_{st}_g{grp}")
                    nc.sync.dma_start(t[:, 0:M], dram[r0 : r0 + PG, 0:M])
                    strips[st] = t
            return strips

        def emit_group(grp, strips, deferred_prev, fine=False):
            """Emit transposes/matmuls/PSUM-drains for grp, interleaving the
            PREVIOUS group's deferred halo MACs + stores through the quad
            loop (so DVE never runs a solid MAC block that stalls PE, and
            stores issue as soon as their half is patched). Returns this
            group's deferred op list."""
            r0 = grp * PG
            out_halves = [
                outp.tile([PG, 2 * HALF], IO_DT, tag="out", name=f"out_g{grp}h{i}")
                for i in range(2)
            ]

            if ablate == "dma":
                for op in deferred_prev:
                    op()
                for hh in range(2):
                    nc.vector.tensor_copy(
                        out=out_halves[hh][:, 0:1],
                        in_=strips["d"][:, hh * HALF : hh * HALF + 1],
                    )
                    store_eng.dma_start(
                        out_dram[r0 : r0 + PG, hh * 2 * HALF : (hh + 1) * 2 * HALF],
                        out_halves[hh][:],
                    )
                return []

            quads = {"d": [], "a": []}

            def make_quad(st, q):
                blocks = [4 * q + i for i in range(4)]
                pt = ps_t.tile([128, 512], pt_dt, tag="ps_t", name=f"pt_{st}{q}")
                for i, b in enumerate(blocks):
                    nc.tensor.transpose(
                        pt[:, 128 * i : 128 * (i + 1)],
                        strips[st][:, b * BLK : (b + 1) * BLK],
                        ident_s,
                    )
                qt = tq.tile([128, 512], mm_dt, tag=f"tq_{st}", name=f"qt_{st}{q}")
                nc.vector.tensor_copy(out=qt[:], in_=pt[:])
                quads[st].append(qt)

            def make_chunk_pair(t):
                # chunks k=2t, 2t+1 share one PSUM bank and one ACT copy
                po = ps_o.tile([128, 2 * OUTW], F32, tag="ps_o", name=f"po_{t}")
                for half_idx in range(2):
                    k = 2 * t + half_idx
                    q, off = divmod(k, 4)
                    lhs_d = quads["d"][q][:, off * 128 : off * 128 + 128]
                    lhs_a = quads["a"][q][:, off * 128 : off * 128 + 128]
                    sl = po[:, half_idx * OUTW : (half_idx + 1) * OUTW]
                    nc.tensor.matmul(sl, lhs_d, wd_s, start=True, stop=False,
                                     skip_group_check=True)
                    nc.tensor.matmul(sl, lhs_a, wa_s, start=False, stop=True,
                                     skip_group_check=True)
                hh, tt = divmod(t, NBLK_HALF // 2)
                nc.scalar.copy(
                    out=out_halves[hh][:, tt * 2 * OUTW : (tt + 1) * 2 * OUTW],
                    in_=po[:],
                )

            # --- halo via compact PATCH tiles --------------------------------
            # patch[p, c, j] = sum over (stream, kp) of x[p, 128(c+1)+kp] *
            # hv2[st,kp,j] for output column n = 249+j of chunk c. Built with
            # broadcast (0-stride) tensor_tensor pairs: one mult over
            # [c=32, j<=7] per (stream, kp) + one accumulate — ~15 DVE ops
            # per half instead of 96 single-column strided MACs (whose
            # per-op cost measured ~26us exposed on HW). Patch-building
            # depends ONLY on input strips so it hides anywhere in the group;
            # just the two final adds touch the strided output view.
            NJ = 7  # patched output columns per chunk: n in [249, 255]

            patches = {}
            xhs = {}

            def halo_build_ops():
                """Ops needing only the input strips; spread through this
                group's own quad loop."""
                if ablate == "nohalo":
                    return []
                ops = []
                for st in ("d", "a"):
                    def wrap(st=st):
                        nc.vector.tensor_copy(
                            out=strips[st][:, M : M + 4], in_=strips[st][:, 0:4]
                        )
                    ops.append(wrap)
                for hh in range(2):
                    for st in ("d", "a"):
                        xh = halo.tile(
                            [128, NBLK_HALF * 4], IO_DT, tag=f"xh_{st}{hh}",
                            name=f"xh_{st}{hh}_g{grp}",
                        )
                        xhs[(st, hh)] = xh

                        def gather(st=st, hh=hh, xh=xh):
                            nc.vector.tensor_copy(
                                out=xh[:].rearrange("p (c k) -> p c k", k=4),
                                in_=strips[st][:, hh * HALF : hh * HALF + EXTW]
                                .rearrange("p (c w) -> p c w", w=BLK)
                                [:, 1 : NBLK_HALF + 1, 0:4],
                            )
                        ops.append(gather)
                    p = halo.tile(
                        [128, NBLK_HALF * NJ], IO_DT, tag=f"patch{hh}",
                        name=f"patch{hh}_g{grp}",
                    )
                    patches[hh] = p
                    sc = halo.tile(
                        [128, NBLK_HALF * NJ], IO_DT, tag=f"scr{hh}",
                        name=f"scr{hh}_g{grp}",
                    )

                    def contrib(hh=hh, st=None, kp=None, p=p, sc=sc, first=False):
                        # dst[:, c, 2kp:7] (+)= xh[:, c, kp] * hv2[st, kp, j]
                        L = NJ - 2 * kp
                        xv = xhs[(st, hh)][:].rearrange("p (c k) -> p c k", k=4)
                        in0 = xv[:, 0:NBLK_HALF, kp : kp + 1]
                        sti = 0 if st == "d" else 1
                        cv = hv2_s[:, (sti * 4 + kp) * NJ + 2 * kp :
                                   (sti * 4 + kp + 1) * NJ]
                        in1 = cv.rearrange("p (c j) -> p c j", c=1)
                        pr = p[:].rearrange("p (c j) -> p c j", j=NJ)
                        dst = pr[:, :, 2 * kp : NJ]
                        b0, b1 = bass.broadcast_tensor_aps(in0, in1)
                        if first:
                            nc.vector.tensor_tensor(
                                out=dst, in0=b0, in1=b1,
                                op=mybir.AluOpType.mult,
                            )
                        else:
                            scr = sc[:].rearrange("p (c j) -> p c j", j=NJ)
                            tmp = scr[:, :, 0:L]
                            nc.vector.tensor_tensor(
                                out=tmp, in0=b0, in1=b1,
                                op=mybir.AluOpType.mult,
                            )
                            nc.vector.tensor_tensor(
                                out=dst, in0=dst, in1=tmp,
                                op=mybir.AluOpType.add,
                            )

                    for sti, st in enumerate(("d", "a")):
                        for kp in range(4):
                            first = (sti == 0 and kp == 0)
                            def op(hh=hh, st=st, kp=kp, p=p, sc=sc, first=first):
                                contrib(hh=hh, st=st, kp=kp, p=p, sc=sc,
                                        first=first)
                            ops.append(op)
                return ops

            def make_deferred():
                """Tail ops: final patch adds (after the half's ACT copies)
                and stores. Fine mode (last group) shrinks the chunks so the
                final copy->add->store drain chain is short."""
                ops = []

                def add_op(hh, c0, c1):
                    p = patches[hh]

                    def op():
                        oh3 = out_halves[hh][:].rearrange(
                            "p (c w) -> p c w", w=OUTW
                        )
                        o = oh3[:, c0:c1, 249 : 249 + NJ]
                        pt = p[:].rearrange("p (c j) -> p c j", j=NJ)
                        nc.vector.tensor_tensor(
                            out=o,
                            in0=pt[:, c0:c1, :],
                            in1=o,
                            op=mybir.AluOpType.add,
                        )
                    return op

                def store_op(hh, c0, c1):
                    def op():
                        store_eng.dma_start(
                            out_dram[
                                r0 : r0 + PG,
                                hh * 2 * HALF + c0 * OUTW : hh * 2 * HALF
                                + c1 * OUTW,
                            ],
                            out_halves[hh][:, c0 * OUTW : c1 * OUTW],
                        )
                    return op

                if fine:
                    subs = {0: [(0, 16), (16, 32)], 1: [(0, 16), (16, 24), (24, 32)]}
                else:
                    subs = {0: [(0, 32)], 1: [(0, 32)]}
                for hh in range(2):
                    for c0, c1 in subs[hh]:
                        if ablate != "nohalo":
                            ops.append(add_op(hh, c0, c1))
                        ops.append(store_op(hh, c0, c1))
                return ops

            own_ops = halo_build_ops()
            nsteps = NBLK // 4  # 16 quad steps
            np_prev = len(deferred_prev)
            n_own = len(own_ops)
            for q in range(nsteps):
                make_quad("d", q)
                make_quad("a", q)
                for t in range(2 * q, 2 * q + 2):
                    make_chunk_pair(t)
                # previous group's tail ops first (stores issue early), then
                # this group's halo-build ops
                for op in deferred_prev[
                    np_prev * q // nsteps : np_prev * (q + 1) // nsteps
                ]:
                    op()
                for op in own_ops[n_own * q // nsteps : n_own * (q + 1) // nsteps]:
                    op()

            return make_deferred()

        def emit_all():
            # primed interleave: 2 groups of loads run ahead of compute, and
            # each later group's loads are emitted BEFORE the previous group's
            # stores so a store's sem-wait never head-of-line-blocks a load on
            # the SP queue.
            prime = min(2, ngroups)
            pending = {g: emit_loads(g) for g in range(prime)}
            deferred = []
            for g in range(ngroups):
                # NOTE: loads for g+prime are emitted AFTER emit_group(g) so
                # that group g-1's deferred MACs (emitted inside emit_group(g))
                # are already recorded as consumers of the input tiles that
                # these loads recycle — otherwise the load would skip that WAR
                # dependency and clobber a strip the MACs still read.
                deferred = emit_group(
                    g, pending.pop(g), deferred, fine=(g == ngroups - 1)
                )
                nxt = g + prime
                if nxt < ngroups:
                    pending[nxt] = emit_loads(nxt)
            # drain the last group's halo MACs + stores
            for op in deferred:
                op()

        if repeat > 1:
            with tc.For_i(0, repeat, 1):
                emit_all()
        else:
            emit_all()

    nc.compile()
    _BUILD_CACHE[key] = nc
    return nc


def _make_consts(scaling):
    """Host-side constants keyed by dram tensor name."""
    wd, wa, hvec, hv2 = _build_weights(scaling)
    ident = np.eye(128, dtype=NP_IO)
    if IO16:
        return {"consts": np.concatenate([ident, wd, wa, hvec, hv2], axis=1)}
    return {"w_d": wd, "w_a": wa, "w_hvec": hvec, "w_hv2": hv2, "ident": ident}


def _run(details, approximation, scaling, rows_per_core, core_ids, mm_f32r, **kw):
    consts = _make_consts(scaling)
    nc = _build(rows_per_core, mm_f32r)
    details = np.asarray(details, dtype=NP_IO)
    approximation = np.asarray(approximation, dtype=NP_IO)
    in_maps = []
    for c in core_ids:
        r0 = c * rows_per_core
        m = {
            "details": np.ascontiguousarray(details[r0 : r0 + rows_per_core]),
            "approximation": np.ascontiguousarray(
                approximation[r0 : r0 + rows_per_core]
            ),
        }
        m.update(consts)
        in_maps.append(m)
    res = run_bass_kernel_spmd(nc, in_maps, core_ids=list(range(len(core_ids))), **kw)
    out = np.concatenate([res.results[i]["out"] for i in range(len(core_ids))], axis=0)
    return out, res


def kernel(details, approximation, scaling):
    details = np.asarray(details, dtype=np.float32)
    approximation = np.asarray(approximation, dtype=np.float32)
    scaling = np.asarray(scaling, dtype=np.float32)
    rows_per_core = details.shape[0] // N_CORES
    out, _ = _run(
        details, approximation, scaling, rows_per_core, list(range(N_CORES)),
        MM_F32R,
    )
    return np.asarray(out, dtype=np.float32)
